# revision 47
# baseline (speedup 1.0000x reference)
"""Data2VecVision self-attention Bass kernel for 8 Trainium2 NeuronCores.

Sharding: data-parallel over batch (64 = 8 cores x 8 batches/core).

Per-core design:
  - hidden_states shard transposed on host to hsT [768, 8*197] (fp16) so the
    contraction dim (hidden) lands on SBUF partitions. All matmuls fp16
    (measured ~3e-4 per-matmul rel err); PSUM accumulation fp32.
  - QT/KT computed whole-core as [d_out, s] fp16; V computed in natural
    [s, d_out] layout padded per-head with a ones column so softmax sums
    fall out of the context matmul for free (sums land in column 64 of
    each head's 65-wide slot).
  - scores computed transposed [j, i] so the softmax reduction (over j)
    is the matmul contraction dim -> no on-chip transposes anywhere.
    Head pairs (2c, 2c+1) live at partitions 0-63 / 64-127 of d_out-chunk c
    and run as row-group matmuls; each head's two j-chunks share one
    1-bank PSUM tile [128, 394]. The j=128..196 chunk uses a 69-col K
    stationary (cheaper LDWEIGHTS); PSUM rows 69-127 of that half stay
    stale and are never read by the ctx matmuls.
  - relative-position bias folded in as exp(s+b) = exp(s)*exp(b): ACT does
    exp(scores) straight from PSUM in one op per head; the host-baked
    exp(bias) multiply runs once per head-PAIR (merged 788-col op) on
    DVE (2/3 of pairs) / GpSimd (1/3), with a 2-pair software pipeline
    lead so the mult latency hides behind the next pairs' scores.
  - 1/sqrt(64) folded into Wq/bq on host; V bias bv folded through the
    softmax identity (sum probs == 1) by keeping bv in V.
  - QK projection drains on DVE; PSUM groups rotate over two pools
    (4 banks) for deep matmul pipelining in the projection phase. The
    ACT engine stays free of phase-2 DMA triggers so the exp chain is
    never blocked behind a transfer.
  - context for 3 head-pairs accumulates into one 1-bank PSUM tile
    [128, 390]; normalization is one DVE reciprocal of the 6 sums columns +
    one wide broadcast multiply per (half, i-chunk) written as fp16;
    output DMAs stream on the sync ring, with the last batch's stores
    fanned across sync/scalar/gpsimd so the tail drains in parallel.
  - PE density: V-projection matmul groups are interleaved into the
    attention stream (over the rotated batch order) as gap fillers.
    Input DMAs fan out over five engine queues with the first projection
    group's operands (wq chunk 0 split in two + the six st=0 hs slices)
    each on their own queue so the first matmul starts early.
  - output y is fp16 on-device; host casts to fp32.
"""

import numpy as np

import concourse.bacc as bacc
import concourse.mybir as mybir
import concourse.tile as tile
from concourse.bass_utils import run_bass_kernel_spmd

F32 = mybir.dt.float32
F16 = mybir.dt.float16
AF = mybir.ActivationFunctionType
ALU = mybir.AluOpType

N_CORES = 8
B = 64
NB = B // N_CORES          # batches per core
S = 197
HID = 768
HEADS = 12
D = 64
NHP = HEADS // 2           # head pairs
NCH = HID // 128           # 6 contraction chunks
NST = 4                    # projection s-tiles per core
SW = NB * S // NST         # 394, projection moving width
CORE_S = NB * S            # 1576
JC = [(0, 128), (128, 69)]   # j/i chunk (offset, len)


def _relative_position_index(h, w):
    coords = np.stack(np.meshgrid(np.arange(h), np.arange(w), indexing="ij")).reshape(2, -1)
    rel = coords[:, :, None] - coords[:, None, :]
    rel = rel.transpose(1, 2, 0).astype(np.int64)
    rel[:, :, 0] += h - 1
    rel[:, :, 1] += w - 1
    rel[:, :, 0] *= 2 * w - 1
    area = h * w
    nrd = (2 * h - 1) * (2 * w - 1) + 3
    idx = np.zeros((area + 1, area + 1), dtype=np.int64)
    idx[1:, 1:] = rel.sum(-1)
    idx[0, :] = nrd - 3
    idx[:, 0] = nrd - 2
    idx[0, 0] = nrd - 1
    return idx


def build_nc(reps=1):
    nc = bacc.Bacc("TRN2", target_bir_lowering=False, debug=False)

    hsT_d = nc.dram_tensor("hsT", [NST, NCH, 128, SW], F16, kind="ExternalInput").ap()
    wq_d = nc.dram_tensor("wqT", [NCH, 128, HID], F16, kind="ExternalInput").ap()  # c-major
    wk_d = nc.dram_tensor("wkT", [NCH, 128, HID], F16, kind="ExternalInput").ap()  # c-major
    wv_d = nc.dram_tensor("wvT", [NCH, 128, HID], F16, kind="ExternalInput").ap()
    bq_d = nc.dram_tensor("bqc", [128, NCH], F32, kind="ExternalInput").ap()
    bv_d = nc.dram_tensor("bvb", [128, HID], F32, kind="ExternalInput").ap()
    eb_d = nc.dram_tensor("expb", [HEADS, 128, 2 * S], F16, kind="ExternalInput").ap()
    y_d = nc.dram_tensor("y", [NB, S, HID], F16, kind="ExternalOutput").ap()

    with tile.TileContext(nc) as tc:
        with (
            tc.tile_pool(name="res", bufs=1) as res,
            tc.tile_pool(name="vpad", bufs=NB * 2) as vpad_pool,
            tc.tile_pool(name="et", bufs=6) as et_pool,
            tc.tile_pool(name="em", bufs=6) as em_pool,
            tc.tile_pool(name="rt", bufs=8) as rt_pool,
            tc.tile_pool(name="ot", bufs=8) as ot_pool,
            tc.tile_pool(name="pc", bufs=2, space="PSUM") as pc_ps,
            tc.tile_pool(name="pj", bufs=2, space="PSUM") as pj_ps,
            tc.tile_pool(name="sp", bufs=2, space="PSUM") as sc_ps,
        ):
            hs_sb = res.tile([128, NCH * CORE_S], F16)
            wq_sb = res.tile([128, NCH * HID], F16)
            wk_sb = res.tile([128, NCH * HID], F16)
            wv_sb = res.tile([128, NCH * HID], F16)
            bq_sb = res.tile([128, NCH], F32)
            bv_sb = res.tile([128, HID], F32)
            eb_sb = res.tile([128, HEADS * 2 * S], F16)
            qt_sb = res.tile([128, NCH * CORE_S], F16)
            kt_sb = res.tile([128, NCH * CORE_S], F16)
            vpad = [[vpad_pool.tile([128, HEADS * 65], F16, tag="vp",
                                    name=f"vpad_{b}_{j}") for j in range(2)]
                    for b in range(NB)]

            for _ in range(reps):
                # ---- input DMAs over the three DMA paths (two HWDGE rings:
                # sync + scalar, FIFO each; one SWDGE ring: gpsimd). The
                # first Q group's operands (wq chunk 0 halves + the six
                # dense st=0 hs slices) lead the two HWDGE rings; weight
                # chunks follow in the order the projection consumes them.
                # Fine-grained rotation over the three rings paces best
                # (per-DMA completion receipt is ~1.5us; small transfers on
                # alternating rings overlap those latencies). Late bulk
                # loads (wv/bv/eb) are merged to save trigger time.
                dma_engs = [nc.sync, nc.scalar, nc.gpsimd]

                def dma(i, dst, src):
                    dma_engs[i % 3].dma_start(dst, src)

                def hs_dma(i, st, c):
                    dma(i, hs_sb[:, c * CORE_S + st * SW: c * CORE_S + (st + 1) * SW],
                        hsT_d[st, c])

                nc.sync.dma_start(wq_sb[:, 0:HID // 2], wq_d[0, :, :HID // 2])
                nc.scalar.dma_start(wq_sb[:, HID // 2:HID], wq_d[0, :, HID // 2:])
                nc.sync.dma_start(bq_sb[:], bq_d[:])
                for c in range(NCH):
                    hs_dma(1 + c, 0, c)
                for c in range(1, NCH):
                    dma(c, wq_sb[:, c * HID:(c + 1) * HID], wq_d[c])
                for c in range(NCH):
                    dma(c + 1, wk_sb[:, c * HID:(c + 1) * HID], wk_d[c])
                for st in range(1, NST):
                    for c in range(NCH):
                        hs_dma(c + st, st, c)
                for c in range(NCH):
                    dma(c, wv_sb[:, c * HID:(c + 1) * HID], wv_d[c])
                nc.sync.dma_start(bv_sb[:], bv_d[:])
                for g in range(HEADS):
                    dma(g, eb_sb[:, g * 2 * S:(g + 1) * 2 * S], eb_d[g])

                # ---- QK projections, whole core. PSUM groups rotate over
                # the pj and pc pools (4 banks deep); drains on DVE.
                pi = [0]

                def proj_psum(nm):
                    pool, tg = [(pj_ps, "pj"), (pc_ps, "pc")][pi[0] % 2]
                    t = pool.tile([128, SW], F32, tag=tg, name=nm)
                    pi[0] += 1
                    return t

                for st in range(NST):
                    for c in range(NCH):
                        qp = proj_psum(f"qp_{st}_{c}")
                        for hch in range(NCH):
                            nc.tensor.matmul(
                                qp[:], wq_sb[:, c * HID + hch * 128: c * HID + (hch + 1) * 128],
                                hs_sb[:, hch * CORE_S + st * SW: hch * CORE_S + (st + 1) * SW],
                                start=(hch == 0), stop=(hch == NCH - 1))
                        nc.vector.tensor_scalar_add(
                            qt_sb[:, c * CORE_S + st * SW: c * CORE_S + (st + 1) * SW],
                            qp[:], bq_sb[:, c:c + 1])
                    for c in range(NCH):
                        kp = proj_psum(f"kp_{st}_{c}")
                        for hch in range(NCH):
                            nc.tensor.matmul(
                                kp[:], wk_sb[:, c * HID + hch * 128: c * HID + (hch + 1) * 128],
                                hs_sb[:, hch * CORE_S + st * SW: hch * CORE_S + (st + 1) * SW],
                                start=(hch == 0), stop=(hch == NCH - 1))
                        nc.vector.tensor_copy(
                            kt_sb[:, c * CORE_S + st * SW: c * CORE_S + (st + 1) * SW],
                            kp[:])

                # ---- V projection emitter: first 2 batches upfront, the rest
                # interleaved into the attention stream as PE gap fillers ----
                def emit_v(b, jci, nts=(0, 1)):
                    joff, jlen = JC[jci]
                    vt = vpad[b][jci]
                    if 0 in nts:
                        ones_ap = vt[:jlen].rearrange("p (h c) -> p h c", h=HEADS)[:, :, 64:65]
                        nc.gpsimd.memset(ones_ap, 1.0)
                    scol = b * S + joff
                    for nt, (noff, nlen) in [(n, [(0, 512), (512, 256)][n]) for n in nts]:
                        vp = pj_ps.tile([128, 512], F32, tag="pj",
                                        name=f"vp_{b}_{jci}_{nt}")
                        for c in range(NCH):
                            nc.tensor.matmul(
                                vp[:jlen, :nlen],
                                hs_sb[:, c * CORE_S + scol: c * CORE_S + scol + jlen],
                                wv_sb[:, c * HID + noff: c * HID + noff + nlen],
                                start=(c == 0), stop=(c == NCH - 1))
                        dst = vt[:jlen, nt * 8 * 65:].rearrange(
                            "p (h c) -> p h c", c=65)[:, :nlen // 64, :64]
                        nc.vector.tensor_tensor(
                            out=dst, in0=vp[:jlen, :nlen],
                            in1=bv_sb[:jlen, noff:noff + nlen],
                            op=ALU.add)

                ATTN_ORDER = [6, 7, 0, 1, 2, 3, 4, 5]
                for jci in range(2):
                    emit_v(ATTN_ORDER[0], jci)

                # ---- attention: per batch, two half-groups of 3 head-pairs.
                # Software-pipelined with a 2-pair lead: scores/exp/mult for
                # pairs p+1, p+2 are emitted before the ctx matmuls of pair p
                # so the exp->mult chain never stalls the PE. ctx for 3 pairs
                # accumulates into one 1-bank PSUM tile [128, 390].
                for bk, b in enumerate(ATTN_ORDER):
                    nxt = ATTN_ORDER[bk + 1] if bk + 1 < NB else None
                    ot = [ot_pool.tile([128, HID], F16, tag="ot",
                                       name=f"ot_{b}_{i}") for i in range(2)]
                    def mk_cps(half):
                        return [pc_ps.tile([128, 390], F32, tag="pc",
                                           name=f"cp_{b}_{half}_{i}") for i in range(2)]

                    if True:
                        def emit_front(hp):
                            c = hp
                            col = c * CORE_S + b * S
                            er = et_pool.tile([128, 4 * S], F16, tag="et",
                                              name=f"er_{b}_{hp}")
                            # both heads' scores share one 2-bank PSUM tile
                            # (h at col 0 / 512) so exp runs as ONE ACT op
                            # over a strided [128, 2, 394] view.
                            sp = sc_ps.tile([128, 1024], F32, tag="sp",
                                            name=f"sp_{b}_{hp}")
                            # jci-major emission: the two heads' row-tiles
                            # (T0/T8) alternate so their LDWEIGHTS/matmuls
                            # overlap across tiles.
                            for h in range(2):
                                nc.tensor.matmul(
                                    sp[:, h * 512: h * 512 + S],
                                    kt_sb[h * 64:(h + 1) * 64, col: col + 128],
                                    qt_sb[h * 64:(h + 1) * 64, col: col + S],
                                    start=True, stop=True)
                            for h in range(2):
                                # j=128..196 chunk: 69-col stationary; PSUM rows
                                # 69-127 of this half stay stale and unread.
                                nc.tensor.matmul(
                                    sp[:69, h * 512 + S: h * 512 + 2 * S],
                                    kt_sb[h * 64:(h + 1) * 64, col + 128: col + 197],
                                    qt_sb[h * 64:(h + 1) * 64, col: col + S],
                                    start=True, stop=True)
                            nc.scalar.activation(
                                er[:].rearrange("p (h w) -> p h w", h=2),
                                sp[:].rearrange("p (h w) -> p h w", h=2)[:, :, :2 * S],
                                AF.Exp)
                            et = em_pool.tile([128, 4 * S], F16, tag="em",
                                              name=f"em_{b}_{hp}")
                            mul_eng = nc.gpsimd if hp % 3 == 0 else nc.vector
                            mul_eng.tensor_tensor(
                                out=et[:], in0=er[:],
                                in1=eb_sb[:, hp * 4 * S:(hp + 1) * 4 * S],
                                op=ALU.mult)
                            return et

                        def emit_ctx(cps, half, hpl, et):
                            for ici, (ioff, ilen) in enumerate(JC):
                                for h in range(2):
                                    for jci, (joff, jlen) in enumerate(JC):
                                        nc.tensor.matmul(
                                            cps[ici][:ilen, hpl * 130 + h * 65:
                                                     hpl * 130 + (h + 1) * 65],
                                            et[:jlen, h * 2 * S + jci * S + ioff:
                                               h * 2 * S + jci * S + ioff + ilen],
                                            vpad[b][jci][:jlen,
                                                         ((half * 3 + hpl) * 2 + h) * 65:
                                                         ((half * 3 + hpl) * 2 + h + 1) * 65],
                                            start=(jci == 0), stop=(jci == 1))

                    def emit_norm(cps, half):
                        for ici, (ioff, ilen) in enumerate(JC):
                            r = rt_pool.tile([128, 6], F32, tag="rt",
                                             name=f"r_{b}_{half}_{ici}")
                            sums = cps[ici][:ilen].rearrange(
                                "p (g c) -> p g c", c=65)[:, :, 64:65]
                            nc.vector.reciprocal(r[:ilen], sums)
                            nc.vector.tensor_tensor(
                                out=ot[ici][:ilen, half * 384:(half + 1) * 384]
                                    .rearrange("p (g c) -> p g c", c=64),
                                in0=cps[ici][:ilen].rearrange(
                                    "p (g c) -> p g c", c=65)[:, :, :64],
                                in1=r[:ilen].broadcast_to([ilen, 6, 64]),
                                op=ALU.mult)
                            if bk < NB - 1:
                                out_eng = nc.sync
                            else:
                                # last batch: fan the final stores across all
                                # three rings so the tail drains in parallel
                                out_eng = [nc.sync, nc.scalar, nc.gpsimd,
                                           nc.scalar][half * 2 + ici]
                            out_eng.dma_start(
                                y_d[b, ioff:ioff + ilen, half * 384:(half + 1) * 384],
                                ot[ici][:ilen, half * 384:(half + 1) * 384])

                    if nxt is not None:
                        for half in range(2):
                            cps = mk_cps(half)
                            pend = []
                            for hpl in range(3):
                                et = emit_front(half * 3 + hpl)
                                pend.append((hpl, et))
                                if hpl == 1:
                                    emit_v(nxt, half)
                                if len(pend) == 3:
                                    emit_ctx(cps, half, *pend.pop(0))
                            for hpl, et in pend:
                                emit_ctx(cps, half, hpl, et)
                            emit_norm(cps, half)
                    else:
                        # Last batch: no next-V filler exists, so weave the
                        # second half's score fronts between the first half's
                        # ctx bursts to keep the PE fed while each pair's
                        # exp->mult chain completes.
                        cps0 = mk_cps(0)
                        e0 = [emit_front(hpl) for hpl in range(3)]
                        e1 = []
                        for hpl in range(3):
                            emit_ctx(cps0, 0, hpl, e0[hpl])
                            if hpl < 2:
                                e1.append(emit_front(3 + hpl))
                        emit_norm(cps0, 0)
                        e1.append(emit_front(5))
                        cps1 = mk_cps(1)
                        for hpl in range(3):
                            emit_ctx(cps1, 1, hpl, e1[hpl])
                        emit_norm(cps1, 1)

    nc.compile()
    return nc


_NC_CACHE = {}


def _get_nc(reps=1):
    if reps not in _NC_CACHE:
        _NC_CACHE[reps] = build_nc(reps)
    return _NC_CACHE[reps]


def prep_inputs(hidden_states, Wq, bq, Wk, Wv, bv, bias_table):
    hidden_states = np.asarray(hidden_states, np.float32)
    Wq = np.asarray(Wq, np.float32)
    bq = np.asarray(bq, np.float32)
    Wk = np.asarray(Wk, np.float32)
    Wv = np.asarray(Wv, np.float32)
    bv = np.asarray(bv, np.float32)
    bias_table = np.asarray(bias_table, np.float32)

    def cmajor(wT):
        # [h_in, d_out] -> [c, p, hch*128+col] so one DMA covers one d_out chunk
        return np.ascontiguousarray(
            wT.reshape(NCH, 128, NCH, 128).transpose(2, 1, 0, 3).reshape(NCH, 128, HID))
    wqT = cmajor((Wq / 8.0).T).astype(np.float16)
    wkT = cmajor(Wk.T).astype(np.float16)
    wvT = np.ascontiguousarray(Wv.T).reshape(NCH, 128, HID).astype(np.float16)
    bqc = np.ascontiguousarray((bq / 8.0).astype(np.float32).reshape(NCH, 128).T)
    bvb = np.ascontiguousarray(np.broadcast_to(bv, (128, HID))).astype(np.float32)

    idx = _relative_position_index(14, 14)
    bias_full = bias_table[idx]              # [S, S, HEADS] (i, j, h)
    biasT = bias_full.transpose(2, 1, 0)     # [h, j, i]
    expb = np.zeros((HEADS, 2, 128, S), np.float32)
    for g in range(HEADS):
        for jci, (joff, jlen) in enumerate(JC):
            expb[g, jci, :jlen, :] = np.exp(biasT[g, joff:joff + jlen, :])
    # [HEADS, 2, 128, S] -> p-major [HEADS, 128, 2*S] so each head's exp(bias)
    # table lands in SBUF with one dense DMA.
    expb = np.ascontiguousarray(
        expb.transpose(0, 2, 1, 3).reshape(HEADS, 128, 2 * S)).astype(np.float16)

    shared = {"wqT": wqT, "wkT": wkT, "wvT": wvT, "bqc": bqc, "bvb": bvb,
              "expb": expb}
    in_maps = []
    for c in range(N_CORES):
        hs_c = hidden_states[c * NB:(c + 1) * NB]            # [NB, S, HID]
        hsT = np.ascontiguousarray(hs_c.transpose(2, 0, 1).reshape(HID, CORE_S))
        hsT = hsT.reshape(NCH, 128, NST, SW).transpose(2, 0, 1, 3)
        in_maps.append({"hsT": np.ascontiguousarray(hsT).astype(np.float16),
                        **shared})
    return in_maps


def run(in_maps, reps=1, **kw):
    nc = _get_nc(reps)
    res = run_bass_kernel_spmd(nc, in_maps, core_ids=list(range(N_CORES)), **kw)
    out = np.concatenate([res.results[c]["y"] for c in range(N_CORES)],
                         axis=0).astype(np.float32)
    return out, res


def kernel(hidden_states, Wq, bq, Wk, Wv, bv, bias_table,
           resolution_h=224, resolution_w=224):
    assert int(resolution_h) == 224 and int(resolution_w) == 224, \
        "kernel compiled for 224x224 (window 14x14, S=197)"
    hidden_states = np.asarray(hidden_states)
    assert hidden_states.shape == (B, S, HID), hidden_states.shape
    in_maps = prep_inputs(hidden_states, Wq, bq, Wk, Wv, bv, bias_table)
    return run(in_maps, reps=1)[0]


# revision 50
# speedup vs baseline: 1.0052x; 1.0052x over previous
"""Data2VecVision self-attention Bass kernel for 8 Trainium2 NeuronCores.

Sharding: data-parallel over batch (64 = 8 cores x 8 batches/core).

Per-core design:
  - hidden_states shard transposed on host to hsT [768, 8*197] (fp16) so the
    contraction dim (hidden) lands on SBUF partitions. All matmuls fp16
    (measured ~3e-4 per-matmul rel err); PSUM accumulation fp32.
  - QT/KT computed whole-core as [d_out, s] fp16; V computed in natural
    [s, d_out] layout padded per-head with a ones column so softmax sums
    fall out of the context matmul for free (sums land in column 64 of
    each head's 65-wide slot).
  - scores computed transposed [j, i] so the softmax reduction (over j)
    is the matmul contraction dim -> no on-chip transposes anywhere.
    Head pairs (2c, 2c+1) live at partitions 0-63 / 64-127 of d_out-chunk c
    and run as row-group matmuls; each head's two j-chunks share one
    1-bank PSUM tile [128, 394]. The j=128..196 chunk uses a 69-col K
    stationary (cheaper LDWEIGHTS); PSUM rows 69-127 of that half stay
    stale and are never read by the ctx matmuls.
  - relative-position bias folded in as exp(s+b) = exp(s)*exp(b): ACT does
    exp(scores) straight from PSUM in one op per head; the host-baked
    exp(bias) multiply runs once per head-PAIR (merged 788-col op) on
    DVE (2/3 of pairs) / GpSimd (1/3), with a 2-pair software pipeline
    lead so the mult latency hides behind the next pairs' scores.
  - 1/sqrt(64) folded into Wq/bq on host; V bias bv folded through the
    softmax identity (sum probs == 1) by keeping bv in V.
  - QK projection drains on DVE; PSUM groups rotate over two pools
    (4 banks) for deep matmul pipelining in the projection phase. The
    ACT engine stays free of phase-2 DMA triggers so the exp chain is
    never blocked behind a transfer.
  - context for 3 head-pairs accumulates into one 1-bank PSUM tile
    [128, 390]; normalization is one DVE reciprocal of the 6 sums columns +
    one wide broadcast multiply per (half, i-chunk) written as fp16;
    output DMAs stream on the sync ring, with the last batch's stores
    fanned across sync/scalar/gpsimd so the tail drains in parallel.
  - PE density: V-projection matmul groups are interleaved into the
    attention stream (over the rotated batch order) as gap fillers.
    Input DMAs fan out over five engine queues with the first projection
    group's operands (wq chunk 0 split in two + the six st=0 hs slices)
    each on their own queue so the first matmul starts early.
  - output y is fp16 on-device; host casts to fp32.
"""

import numpy as np

import concourse.bacc as bacc
import concourse.mybir as mybir
import concourse.tile as tile
from concourse.bass_utils import run_bass_kernel_spmd

F32 = mybir.dt.float32
F16 = mybir.dt.float16
AF = mybir.ActivationFunctionType
ALU = mybir.AluOpType

N_CORES = 8
B = 64
NB = B // N_CORES          # batches per core
S = 197
HID = 768
HEADS = 12
D = 64
NHP = HEADS // 2           # head pairs
NCH = HID // 128           # 6 contraction chunks
NST = 4                    # projection s-tiles per core
SW = NB * S // NST         # 394, projection moving width
CORE_S = NB * S            # 1576
JC = [(0, 128), (128, 69)]   # j/i chunk (offset, len)


def _relative_position_index(h, w):
    coords = np.stack(np.meshgrid(np.arange(h), np.arange(w), indexing="ij")).reshape(2, -1)
    rel = coords[:, :, None] - coords[:, None, :]
    rel = rel.transpose(1, 2, 0).astype(np.int64)
    rel[:, :, 0] += h - 1
    rel[:, :, 1] += w - 1
    rel[:, :, 0] *= 2 * w - 1
    area = h * w
    nrd = (2 * h - 1) * (2 * w - 1) + 3
    idx = np.zeros((area + 1, area + 1), dtype=np.int64)
    idx[1:, 1:] = rel.sum(-1)
    idx[0, :] = nrd - 3
    idx[:, 0] = nrd - 2
    idx[0, 0] = nrd - 1
    return idx


def build_nc(reps=1):
    nc = bacc.Bacc("TRN2", target_bir_lowering=False, debug=False)

    hsT_d = nc.dram_tensor("hsT", [NST, NCH, 128, SW], F16, kind="ExternalInput").ap()
    wq_d = nc.dram_tensor("wqT", [NCH, 128, HID], F16, kind="ExternalInput").ap()  # c-major
    wk_d = nc.dram_tensor("wkT", [NCH, 128, HID], F16, kind="ExternalInput").ap()  # c-major
    wv_d = nc.dram_tensor("wvT", [NCH, 128, HID], F16, kind="ExternalInput").ap()
    bq_d = nc.dram_tensor("bqc", [128, NCH], F32, kind="ExternalInput").ap()
    bv_d = nc.dram_tensor("bvb", [128, HID], F32, kind="ExternalInput").ap()
    eb_d = nc.dram_tensor("expb", [HEADS, 128, 2 * S], F16, kind="ExternalInput").ap()
    y_d = nc.dram_tensor("y", [NB, S, HID], F16, kind="ExternalOutput").ap()

    with tile.TileContext(nc) as tc:
        with (
            tc.tile_pool(name="res", bufs=1) as res,
            tc.tile_pool(name="vpad", bufs=NB * 2) as vpad_pool,
            tc.tile_pool(name="et", bufs=6) as et_pool,
            tc.tile_pool(name="em", bufs=6) as em_pool,
            tc.tile_pool(name="rt", bufs=8) as rt_pool,
            tc.tile_pool(name="ot", bufs=8) as ot_pool,
            tc.tile_pool(name="pc", bufs=2, space="PSUM") as pc_ps,
            tc.tile_pool(name="pj", bufs=2, space="PSUM") as pj_ps,
            tc.tile_pool(name="sp", bufs=2, space="PSUM") as sc_ps,
        ):
            hs_sb = res.tile([128, NCH * CORE_S], F16)
            wq_sb = res.tile([128, NCH * HID], F16)
            wk_sb = res.tile([128, NCH * HID], F16)
            wv_sb = res.tile([128, NCH * HID], F16)
            bq_sb = res.tile([128, NCH], F32)
            bv_sb = res.tile([128, HID], F32)
            eb_sb = res.tile([128, HEADS * 2 * S], F16)
            qt_sb = res.tile([128, NCH * CORE_S], F16)
            kt_sb = res.tile([128, NCH * CORE_S], F16)
            vpad = [[vpad_pool.tile([128, HEADS * 65], F16, tag="vp",
                                    name=f"vpad_{b}_{j}") for j in range(2)]
                    for b in range(NB)]

            for _ in range(reps):
                # ---- input DMAs over the three DMA paths (two HWDGE rings:
                # sync + scalar, FIFO each; one SWDGE ring: gpsimd). The
                # first Q group's operands (wq chunk 0 halves + the six
                # dense st=0 hs slices) lead the two HWDGE rings; weight
                # chunks follow in the order the projection consumes them.
                # Fine-grained rotation over the three rings paces best
                # (per-DMA completion receipt is ~1.5us; small transfers on
                # alternating rings overlap those latencies). Late bulk
                # loads (wv/bv/eb) are merged to save trigger time.
                dma_engs = [nc.sync, nc.scalar, nc.gpsimd]

                def dma(i, dst, src):
                    dma_engs[i % 3].dma_start(dst, src)

                def hs_dma(i, st, c):
                    dma(i, hs_sb[:, c * CORE_S + st * SW: c * CORE_S + (st + 1) * SW],
                        hsT_d[st, c])

                nc.sync.dma_start(wq_sb[:, 0:HID // 2], wq_d[0, :, :HID // 2])
                nc.scalar.dma_start(wq_sb[:, HID // 2:HID], wq_d[0, :, HID // 2:])
                nc.sync.dma_start(bq_sb[:], bq_d[:])
                for c in range(NCH):
                    hs_dma(1 + c, 0, c)
                for c in range(1, NCH):
                    dma(c, wq_sb[:, c * HID:(c + 1) * HID], wq_d[c])
                for c in range(NCH):
                    dma(c + 1, wk_sb[:, c * HID:(c + 1) * HID], wk_d[c])
                for st in range(1, NST):
                    for c in range(NCH):
                        hs_dma(c + st, st, c)
                for c in range(NCH):
                    dma(c, wv_sb[:, c * HID:(c + 1) * HID], wv_d[c])
                nc.sync.dma_start(bv_sb[:], bv_d[:])
                for g in range(HEADS):
                    dma(g, eb_sb[:, g * 2 * S:(g + 1) * 2 * S], eb_d[g])

                # ---- QK projections, whole core. PSUM groups rotate over
                # the pj and pc pools (4 banks deep); drains on DVE.
                pi = [0]

                def proj_psum(nm):
                    pool, tg = [(pj_ps, "pj"), (pc_ps, "pc")][pi[0] % 2]
                    t = pool.tile([128, SW], F32, tag=tg, name=nm)
                    pi[0] += 1
                    return t

                for st in range(NST):
                    for c in range(NCH):
                        qp = proj_psum(f"qp_{st}_{c}")
                        for hch in range(NCH):
                            nc.tensor.matmul(
                                qp[:], wq_sb[:, c * HID + hch * 128: c * HID + (hch + 1) * 128],
                                hs_sb[:, hch * CORE_S + st * SW: hch * CORE_S + (st + 1) * SW],
                                start=(hch == 0), stop=(hch == NCH - 1))
                        nc.vector.tensor_scalar_add(
                            qt_sb[:, c * CORE_S + st * SW: c * CORE_S + (st + 1) * SW],
                            qp[:], bq_sb[:, c:c + 1])
                    for c in range(NCH):
                        kp = proj_psum(f"kp_{st}_{c}")
                        for hch in range(NCH):
                            nc.tensor.matmul(
                                kp[:], wk_sb[:, c * HID + hch * 128: c * HID + (hch + 1) * 128],
                                hs_sb[:, hch * CORE_S + st * SW: hch * CORE_S + (st + 1) * SW],
                                start=(hch == 0), stop=(hch == NCH - 1))
                        nc.vector.tensor_copy(
                            kt_sb[:, c * CORE_S + st * SW: c * CORE_S + (st + 1) * SW],
                            kp[:])

                # ---- V projection emitter: first 2 batches upfront, the rest
                # interleaved into the attention stream as PE gap fillers ----
                def emit_v(b, jci, nts=(0, 1)):
                    joff, jlen = JC[jci]
                    vt = vpad[b][jci]
                    if 0 in nts:
                        ones_ap = vt[:jlen].rearrange("p (h c) -> p h c", h=HEADS)[:, :, 64:65]
                        nc.gpsimd.memset(ones_ap, 1.0)
                    scol = b * S + joff
                    for nt, (noff, nlen) in [(n, [(0, 512), (512, 256)][n]) for n in nts]:
                        vp = pj_ps.tile([128, 512], F32, tag="pj",
                                        name=f"vp_{b}_{jci}_{nt}")
                        for c in range(NCH):
                            nc.tensor.matmul(
                                vp[:jlen, :nlen],
                                hs_sb[:, c * CORE_S + scol: c * CORE_S + scol + jlen],
                                wv_sb[:, c * HID + noff: c * HID + noff + nlen],
                                start=(c == 0), stop=(c == NCH - 1))
                        dst = vt[:jlen, nt * 8 * 65:].rearrange(
                            "p (h c) -> p h c", c=65)[:, :nlen // 64, :64]
                        nc.vector.tensor_tensor(
                            out=dst, in0=vp[:jlen, :nlen],
                            in1=bv_sb[:jlen, noff:noff + nlen],
                            op=ALU.add)

                ATTN_ORDER = [6, 7, 0, 1, 2, 3, 4, 5]
                for jci in range(2):
                    emit_v(ATTN_ORDER[0], jci)

                # ---- attention: per batch, two half-groups of 3 head-pairs.
                # Software-pipelined with a 2-pair lead: scores/exp/mult for
                # pairs p+1, p+2 are emitted before the ctx matmuls of pair p
                # so the exp->mult chain never stalls the PE. ctx for 3 pairs
                # accumulates into one 1-bank PSUM tile [128, 390].
                for bk, b in enumerate(ATTN_ORDER):
                    nxt = ATTN_ORDER[bk + 1] if bk + 1 < NB else None
                    ot = [ot_pool.tile([128, HID], F16, tag="ot",
                                       name=f"ot_{b}_{i}") for i in range(2)]
                    for half in range(2):
                        cps = [pc_ps.tile([128, 390], F32, tag="pc",
                                          name=f"cp_{b}_{half}_{i}") for i in range(2)]

                        def emit_front(hp):
                            c = hp
                            col = c * CORE_S + b * S
                            er = et_pool.tile([128, 4 * S], F16, tag="et",
                                              name=f"er_{b}_{hp}")
                            # both heads' scores share one 2-bank PSUM tile
                            # (h at col 0 / 512) so exp runs as ONE ACT op
                            # over a strided [128, 2, 394] view.
                            sp = sc_ps.tile([128, 1024], F32, tag="sp",
                                            name=f"sp_{b}_{hp}")
                            # jci-major emission: the two heads' row-tiles
                            # (T0/T8) alternate so their LDWEIGHTS/matmuls
                            # overlap across tiles.
                            for h in range(2):
                                # j=128..196 chunk first (69-col stationary =
                                # cheap exposed LDWEIGHTS after the preceding
                                # ctx burst); PSUM rows 69-127 of this half
                                # stay stale and unread.
                                nc.tensor.matmul(
                                    sp[:69, h * 512 + S: h * 512 + 2 * S],
                                    kt_sb[h * 64:(h + 1) * 64, col + 128: col + 197],
                                    qt_sb[h * 64:(h + 1) * 64, col: col + S],
                                    start=True, stop=True)
                            for h in range(2):
                                nc.tensor.matmul(
                                    sp[:, h * 512: h * 512 + S],
                                    kt_sb[h * 64:(h + 1) * 64, col: col + 128],
                                    qt_sb[h * 64:(h + 1) * 64, col: col + S],
                                    start=True, stop=True)
                            nc.scalar.activation(
                                er[:].rearrange("p (h w) -> p h w", h=2),
                                sp[:].rearrange("p (h w) -> p h w", h=2)[:, :, :2 * S],
                                AF.Exp)
                            et = em_pool.tile([128, 4 * S], F16, tag="em",
                                              name=f"em_{b}_{hp}")
                            mul_eng = nc.gpsimd if hp % 3 == 0 else nc.vector
                            mul_eng.tensor_tensor(
                                out=et[:], in0=er[:],
                                in1=eb_sb[:, hp * 4 * S:(hp + 1) * 4 * S],
                                op=ALU.mult)
                            return et

                        def emit_ctx(hpl, et):
                            # i-chunk 1 first: its 69-col stationaries make
                            # the burst's exposed lead-in LDWEIGHTS cheap.
                            for ici, (ioff, ilen) in ((1, JC[1]), (0, JC[0])):
                                for h in range(2):
                                    for jci, (joff, jlen) in enumerate(JC):
                                        nc.tensor.matmul(
                                            cps[ici][:ilen, hpl * 130 + h * 65:
                                                     hpl * 130 + (h + 1) * 65],
                                            et[:jlen, h * 2 * S + jci * S + ioff:
                                               h * 2 * S + jci * S + ioff + ilen],
                                            vpad[b][jci][:jlen,
                                                         ((half * 3 + hpl) * 2 + h) * 65:
                                                         ((half * 3 + hpl) * 2 + h + 1) * 65],
                                            start=(jci == 0), stop=(jci == 1))

                        pend = []
                        for hpl in range(3):
                            et = emit_front(half * 3 + hpl)
                            pend.append((hpl, et))
                            if hpl == 1 and nxt is not None:
                                emit_v(nxt, half)
                            if len(pend) == 3:
                                emit_ctx(*pend.pop(0))
                        for hpl, et in pend:
                            emit_ctx(hpl, et)

                        for ici, (ioff, ilen) in enumerate(JC):
                            r = rt_pool.tile([128, 6], F32, tag="rt",
                                             name=f"r_{b}_{half}_{ici}")
                            sums = cps[ici][:ilen].rearrange(
                                "p (g c) -> p g c", c=65)[:, :, 64:65]
                            nc.vector.reciprocal(r[:ilen], sums)
                            nc.vector.tensor_tensor(
                                out=ot[ici][:ilen, half * 384:(half + 1) * 384]
                                    .rearrange("p (g c) -> p g c", c=64),
                                in0=cps[ici][:ilen].rearrange(
                                    "p (g c) -> p g c", c=65)[:, :, :64],
                                in1=r[:ilen].broadcast_to([ilen, 6, 64]),
                                op=ALU.mult)
                            if bk < NB - 1:
                                out_eng = nc.sync
                            else:
                                # last batch: fan the final stores across all
                                # three rings so the tail drains in parallel
                                out_eng = [nc.sync, nc.scalar, nc.gpsimd,
                                           nc.scalar][half * 2 + ici]
                            out_eng.dma_start(
                                y_d[b, ioff:ioff + ilen, half * 384:(half + 1) * 384],
                                ot[ici][:ilen, half * 384:(half + 1) * 384])

    nc.compile()
    return nc


_NC_CACHE = {}


def _get_nc(reps=1):
    if reps not in _NC_CACHE:
        _NC_CACHE[reps] = build_nc(reps)
    return _NC_CACHE[reps]


def prep_inputs(hidden_states, Wq, bq, Wk, Wv, bv, bias_table):
    hidden_states = np.asarray(hidden_states, np.float32)
    Wq = np.asarray(Wq, np.float32)
    bq = np.asarray(bq, np.float32)
    Wk = np.asarray(Wk, np.float32)
    Wv = np.asarray(Wv, np.float32)
    bv = np.asarray(bv, np.float32)
    bias_table = np.asarray(bias_table, np.float32)

    def cmajor(wT):
        # [h_in, d_out] -> [c, p, hch*128+col] so one DMA covers one d_out chunk
        return np.ascontiguousarray(
            wT.reshape(NCH, 128, NCH, 128).transpose(2, 1, 0, 3).reshape(NCH, 128, HID))
    wqT = cmajor((Wq / 8.0).T).astype(np.float16)
    wkT = cmajor(Wk.T).astype(np.float16)
    wvT = np.ascontiguousarray(Wv.T).reshape(NCH, 128, HID).astype(np.float16)
    bqc = np.ascontiguousarray((bq / 8.0).astype(np.float32).reshape(NCH, 128).T)
    bvb = np.ascontiguousarray(np.broadcast_to(bv, (128, HID))).astype(np.float32)

    idx = _relative_position_index(14, 14)
    bias_full = bias_table[idx]              # [S, S, HEADS] (i, j, h)
    biasT = bias_full.transpose(2, 1, 0)     # [h, j, i]
    expb = np.zeros((HEADS, 2, 128, S), np.float32)
    for g in range(HEADS):
        for jci, (joff, jlen) in enumerate(JC):
            expb[g, jci, :jlen, :] = np.exp(biasT[g, joff:joff + jlen, :])
    # [HEADS, 2, 128, S] -> p-major [HEADS, 128, 2*S] so each head's exp(bias)
    # table lands in SBUF with one dense DMA.
    expb = np.ascontiguousarray(
        expb.transpose(0, 2, 1, 3).reshape(HEADS, 128, 2 * S)).astype(np.float16)

    shared = {"wqT": wqT, "wkT": wkT, "wvT": wvT, "bqc": bqc, "bvb": bvb,
              "expb": expb}
    in_maps = []
    for c in range(N_CORES):
        hs_c = hidden_states[c * NB:(c + 1) * NB]            # [NB, S, HID]
        hsT = np.ascontiguousarray(hs_c.transpose(2, 0, 1).reshape(HID, CORE_S))
        hsT = hsT.reshape(NCH, 128, NST, SW).transpose(2, 0, 1, 3)
        in_maps.append({"hsT": np.ascontiguousarray(hsT).astype(np.float16),
                        **shared})
    return in_maps


def run(in_maps, reps=1, **kw):
    nc = _get_nc(reps)
    res = run_bass_kernel_spmd(nc, in_maps, core_ids=list(range(N_CORES)), **kw)
    out = np.concatenate([res.results[c]["y"] for c in range(N_CORES)],
                         axis=0).astype(np.float32)
    return out, res


def kernel(hidden_states, Wq, bq, Wk, Wv, bv, bias_table,
           resolution_h=224, resolution_w=224):
    assert int(resolution_h) == 224 and int(resolution_w) == 224, \
        "kernel compiled for 224x224 (window 14x14, S=197)"
    hidden_states = np.asarray(hidden_states)
    assert hidden_states.shape == (B, S, HID), hidden_states.shape
    in_maps = prep_inputs(hidden_states, Wq, bq, Wk, Wv, bv, bias_table)
    return run(in_maps, reps=1)[0]
